# revision 1
# baseline (speedup 1.0000x reference)
"""Trainium2 Bass kernel for nn_CrossModalAttention.

Math: the reference broadcasts `language` across the T axis before the
k/v projections, so every key row (and value row) within a batch is
identical.  Attention scores are therefore constant along the key axis,
softmax over a constant vector is exactly uniform (max-subtraction gives
exp(0)=1 for every entry, sum=T, each weight exactly 1/T), and the
attention context collapses to the (identical) value row itself.  The
q/k paths cancel out of the output entirely.  What remains per batch b:

    row_b = (((language_b @ Wv + bv) @ Wv2 + bv2) @ Wo + bo) @ Wout + bout
    out_b = state_b + row_b[None, :]          # broadcast over T

The weight chain is input-independent, so it is constant-folded on the
host (exact distributivity):

    W_eff = Wv @ Wv2 @ Wo @ Wout                      [768, 384]
    b_eff = ((bv @ Wv2 + bv2) @ Wo + bo) @ Wout + bout
    row_b = language_b @ W_eff + b_eff

On device (per core, data-parallel over batch B=8 across 8 cores):
language is replicated across all 128 PE columns (per-partition
tensor_scalar broadcast on DVE), so a single 7-chunk K-accumulated
fp32 matmul produces row_b already broadcast to [128, 384] in PSUM
(chunk 7 is the e0/bias-fold chunk).  VectorE then streams
state + row -> out.  All large tensors are pre-transposed on the host
into partition-major [128, cols] layout so every DMA is a contiguous
2D copy with multi-KB descriptors (near line-rate), and the kernel is
HBM-bound at ~4.6 MB/core of DMA traffic.

Written in raw Bass (explicit per-engine programs + semaphores): the
walrus build here accepts only one sync-wait per TPB instruction, so
Tile's fused-wait scheduling cannot compile; standalone wait_ge
instructions always carry exactly one condition.
"""

from contextlib import ExitStack

import numpy as np

import concourse.bass as bass
import concourse.mybir as mybir
from concourse.bass_utils import run_bass_kernel_spmd

B, T, D = 8, 1024, 384
DL, H = 768, 512
P = 128
KC = DL // P + 1       # 7 chunks: 6 language + 1 bias (e0 fold)
WG = [(0, 2), (2, 7)]  # weff DMA groups (pipelined receipts)
NT = T // P            # 8 t-tiles
NSC = 2                # state load chunks
TPC = NT // NSC        # t-tiles per load chunk
OSPL = [(0, 3), (3, 6), (6, 8)]  # out chunks: one per ring (ACT/SWDGE/SP)
SW = NT * D            # state/out width in partition-major layout (3072)
CW = TPC * D           # chunk width (768)
F32 = mybir.dt.float32

LAST_RESULTS = None  # BassKernelResults of the most recent run (for test.py)


def _build():
    nc = bass.Bass("TRN2", enable_partition_id=False)

    # all partition-major, host-pretransposed:
    #   state[p, n*D+d]  = state_full[n*128+p, d]
    #   weff[p, c*D+m]   = W_eff_aug[c*128+p, m]
    #   langc[:, 0:6] = language chunks (column layout), langc[:, 6] = e0
    state = nc.dram_tensor("state", [P, SW], F32, kind="ExternalInput")
    langc = nc.dram_tensor("langc", [P, KC], F32, kind="ExternalInput")
    weff = nc.dram_tensor("weff", [P, KC * D], F32, kind="ExternalInput")
    out = nc.dram_tensor("out", [P, SW], F32, kind="ExternalOutput")

    with ExitStack() as ctx:
        e = ctx.enter_context
        s_par = e(nc.semaphore("s_par"))
        s_w = [e(nc.semaphore(f"s_w{i}")) for i in range(len(WG))]
        s_stc = [e(nc.semaphore(f"s_st{i}")) for i in range(NSC)]
        s_out = e(nc.semaphore("s_out"))
        pe_sem = e(nc.semaphore("pe_sem"))
        v_sem = e(nc.semaphore("v_sem"))
        lc = e(nc.sbuf_tensor("lc_t", [P, KC], F32))
        ws = e(nc.sbuf_tensor("w_t", [P, KC * D], F32))
        lrep = e(nc.sbuf_tensor("lrep_t", [P, KC * P], F32))
        ones = e(nc.sbuf_tensor("ones_t", [P, P], F32))
        st = e(nc.sbuf_tensor("st_t", [P, SW], F32))
        ob = e(nc.sbuf_tensor("ob_t", [P, SW], F32))
        psb = e(nc.psum_tensor("psb_t", [P, D], F32))
        scr = e(nc.psum_tensor("scr_t", [P, 512], F32))
        block = e(nc.Block())

        @block.sync
        def _(sync):
            # one ring, FIFO-ordered: weff gets full bandwidth first, the
            # state chunks queue right behind it
            sync.dma_start(lc[:, :], langc[:, :]).then_inc(s_par, 16)
            for g, (k0, k1) in enumerate(WG):
                sync.dma_start(ws[:, k0 * D:k1 * D],
                               weff[:, k0 * D:k1 * D]).then_inc(s_w[g], 16)
            for c in range(NSC):
                sync.dma_start(
                    st[:, c * CW:(c + 1) * CW],
                    state[:, c * CW:(c + 1) * CW],
                ).then_inc(s_stc[c], 16)
            # last (smallest) output store on this ring
            sync.wait_ge(v_sem, 5)
            sync.dma_start(out[:, OSPL[2][0] * D:SW],
                           ob[:, OSPL[2][0] * D:SW]).then_inc(s_out, 16)
            sync.wait_ge(s_out, 3 * 16)

        @block.scalar
        def _(scalar):
            # first output store on the ACT HWDGE ring, parallel to loads
            scalar.wait_ge(v_sem, 3)
            scalar.dma_start(out[:, 0:OSPL[0][1] * D],
                             ob[:, 0:OSPL[0][1] * D]).then_inc(s_out, 16)

        @block.gpsimd
        def _(gpsimd):
            # middle output store via SWDGE (third independent ring)
            gpsimd.wait_ge(v_sem, 4)
            gpsimd.dma_start(out[:, OSPL[1][0] * D:OSPL[1][1] * D],
                             ob[:, OSPL[1][0] * D:OSPL[1][1] * D]).then_inc(s_out, 16)

        @block.tensor
        def _(tensor):
            tensor.wait_ge(v_sem, 1)        # ones ready
            # warm the PE HAM clock gate while DMAs stream (~4us of
            # high-duty-cycle dummy matmuls on garbage SBUF; cold PE runs
            # at 1.2 GHz, warm at 2.4 GHz)
            for _ in range(4):
                tensor.matmul(scr[:, :], lhsT=ones[:, :], rhs=lrep[:, 0:512],
                              start=True, stop=True)
            tensor.wait_ge(v_sem, 2)        # langrep ready
            for g, (k0, k1) in enumerate(WG):
                tensor.wait_ge(s_w[g], 16)
                for kc in range(k0, k1):
                    mm = tensor.matmul(
                        psb[:, :],
                        lhsT=lrep[:, kc * P:(kc + 1) * P],
                        rhs=ws[:, kc * D:(kc + 1) * D],
                        start=(kc == 0), stop=(kc == KC - 1),
                    )
            mm.then_inc(pe_sem)             # pe=1: broadcast row in PSUM

        @block.vector
        def _(vector):
            # replicate language across PE columns: lrep[k, m] = lang[k]
            vector.memset(ones[:, :], 1.0).then_inc(v_sem)     # v=1
            vector.wait_ge(s_par, 16)
            for kc in range(KC):
                ts = vector.tensor_scalar_mul(
                    lrep[:, kc * P:(kc + 1) * P], ones[:, :], lc[:, kc:kc + 1]
                )
            ts.then_inc(v_sem)              # v=2
            vector.wait_ge(pe_sem, 1)
            vector.wait_ge(s_stc[0], 16)    # tiles 0-3
            done_st1 = False
            for g, (n0, n1) in enumerate(OSPL):
                for n in range(n0, n1):
                    if n >= NT // 2 and not done_st1:
                        vector.wait_ge(s_stc[1], 16)   # tiles 4-7
                        done_st1 = True
                    a = vector.tensor_add(ob[:, n * D:(n + 1) * D],
                                          st[:, n * D:(n + 1) * D], psb[:, :])
                a.then_inc(v_sem)           # v=3+g

    return nc


def kernel(**inputs) -> np.ndarray:
    global LAST_RESULTS
    f = np.float32
    state = np.asarray(inputs["state"], dtype=f)
    language = np.ascontiguousarray(np.asarray(inputs["language"], dtype=f))
    Wv = np.asarray(inputs["Wv"], dtype=f)
    bv = np.asarray(inputs["bv"], dtype=f)
    Wv2 = np.asarray(inputs["Wv2"], dtype=f)
    bv2 = np.asarray(inputs["bv2"], dtype=f)
    Wo = np.asarray(inputs["Wo"], dtype=f)
    bo = np.asarray(inputs["bo"], dtype=f)
    Wout = np.asarray(inputs["Wout"], dtype=f)
    bout = np.asarray(inputs["bout"], dtype=f)

    # constant-fold the weight chain (input-independent)
    w_eff = ((Wv @ Wv2) @ Wo) @ Wout                      # [768, 384]
    b_eff = ((bv @ Wv2 + bv2) @ Wo + bo) @ Wout + bout    # [384]
    weff_aug = np.zeros((KC * P, D), dtype=f)
    weff_aug[:DL] = w_eff
    weff_aug[DL] = b_eff
    # partition-major: weff_t[p, c*D+m] = weff_aug[c*128+p, m]
    weff_t = np.ascontiguousarray(
        weff_aug.reshape(KC, P, D).transpose(1, 0, 2).reshape(P, KC * D))

    nc = _build()
    in_maps = []
    for b in range(B):
        lcv = np.zeros((P, KC), dtype=f)
        lcv[:, :DL // P] = language[b].reshape(DL // P, P).T
        lcv[0, DL // P] = 1.0
        st_t = np.ascontiguousarray(
            state[b].reshape(NT, P, D).transpose(1, 0, 2).reshape(P, SW))
        in_maps.append({"state": st_t, "langc": lcv, "weff": weff_t})

    res = run_bass_kernel_spmd(nc, in_maps, core_ids=list(range(B)))
    LAST_RESULTS = res
    # un-transpose: out_full[b][n*128+p, d] = out_core[p, n*D+d]
    return np.stack(
        [res.results[b]["out"].reshape(P, NT, D).transpose(1, 0, 2)
         .reshape(T, D) for b in range(B)],
        axis=0)

